# revision 5
# baseline (speedup 1.0000x reference)
"""FDK cone-beam forward projector on 8 trn2 NeuronCores (Bass).

Sharding: angle axis (64 angles -> 8 per core), volume replicated.

Device pipeline (two Bass launches, all arithmetic on device):
  launch 1 (per angle): 4-tap bilinear rotation reduction on DVE:
      rv[k,x,z] = sum_st w1[st,k,x] * G1[k, slot(x,st), z]
  launch 2 (per angle): detector x-interp on DVE + z-interp / y-sum on PE
      (PSUM-accumulated matmuls) + DIST scale.
The bilinear tap index tables are precomputed on host in float32 arithmetic
mirroring the reference exactly; the host applies the (pure data-movement)
tap gathers when staging each launch's inputs.
"""
import sys

sys.path.insert(0, "/opt/trn_rl_repo")

import numpy as np
import concourse.bass as bass
import concourse.bacc as bacc
import concourse.mybir as mybir
from concourse.bass_utils import run_bass_kernel_spmd
from concourse.tile import TileContext

# ---- geometry constants (mirror reference) ----
NA = 64
NZ, NY, NX = 8, 256, 256
NU, NV = 512, 8
DSD, DSO = 1085.6, 595.0
FOV, DZ = 500.0, 1.0
DU, DV = 1.0, 1.0
DX = DY = FOV / NX
HSX = DX * (NX / 2 - 0.5)
HSY = DY * (NY / 2 - 0.5)
HSZ = DZ * (NZ / 2 - 0.5)
ANGLES = np.arange(NA, dtype=np.float64) * (2.0 * np.pi / NA)
XS = (np.arange(NX) - NX / 2 + 0.5) * DX
YS = (np.arange(NY) - NY / 2 + 0.5) * DY
US = (np.arange(NU) - NU / 2 + 0.5) * DU
VS = (np.arange(NV) - NV / 2 + 0.5) * DV
XXN, YYN = np.meshgrid(XS / HSX, YS / HSY)
UU, VV = np.meshgrid(US, VS)
RATIO = (DSO - YS) / DSD
PU = UU[None] * RATIO[:, None, None] / HSX
PV = VV[None] * RATIO[:, None, None] / HSZ
DIST = (np.sqrt(DSD ** 2 + UU ** 2 + VV ** 2) / DSD * DY).astype(np.float32)

N_CORES = 8
A_PER_CORE = NA // N_CORES
f32 = np.float32


def _host_tables():
    cs_all = f32(np.cos(ANGLES))
    sn_all = f32(np.sin(ANGLES))
    xxn = XXN.astype(f32)
    yyn = YYN.astype(f32)
    idx1 = np.zeros((NA, NY, 4 * NX), dtype=np.int64)   # [a, k, slot=4x+st]
    w1 = np.zeros((NA, NY, 4, NX), dtype=f32)           # [a, k, st, x]
    for a in range(NA):
        cs, sn = -sn_all[a], cs_all[a]
        rx = (-xxn * sn + yyn * cs).astype(f32)
        ry = (xxn * cs + yyn * sn).astype(f32)
        xpix = ((rx + f32(1.0)) * f32(0.5) * f32(NX - 1)).astype(f32)
        ypix = ((ry + f32(1.0)) * f32(0.5) * f32(NY - 1)).astype(f32)
        x0f = np.floor(xpix)
        y0f = np.floor(ypix)
        wx1 = (xpix - x0f).astype(f32)
        wy1 = (ypix - y0f).astype(f32)
        x0 = x0f.astype(np.int64)
        y0 = y0f.astype(np.int64)
        wx0 = (f32(1.0) - wx1).astype(f32)
        wy0 = (f32(1.0) - wy1).astype(f32)
        for s in range(2):
            yi = y0 + s
            oky = (yi >= 0) & (yi < NY)
            yc = np.clip(yi, 0, NY - 1)
            wys = wy0 if s == 0 else wy1
            for t in range(2):
                xi = x0 + t
                okx = (xi >= 0) & (xi < NX)
                xc = np.clip(xi, 0, NX - 1)
                wxs = wx0 if t == 0 else wx1
                st = 2 * s + t
                w1[a, :, st, :] = (wys * wxs).astype(f32) * (oky & okx).astype(f32)
                idx1[a, :, st::4] = yc * NX + xc
    # stage 2 (angle-independent)
    pu = PU[:, 0, :].astype(f32)
    xpix2 = ((pu + f32(1.0)) * f32(0.5) * f32(NX - 1)).astype(f32)
    x20f = np.floor(xpix2)
    wx21 = (xpix2 - x20f).astype(f32)
    x20 = x20f.astype(np.int64)
    assert x20.min() >= 0 and x20.max() + 1 <= NX - 1
    wx20 = (f32(1.0) - wx21).astype(f32)
    idx2 = np.zeros((NY, 2 * NU), dtype=np.int64)       # [k, slot=2u+t]
    kg = np.arange(NY)[:, None]
    idx2[:, 0::2] = kg * NX + x20
    idx2[:, 1::2] = kg * NX + x20 + 1
    w2 = np.stack([wx20, wx21], axis=2)                 # [k, u, t]
    pv = PV[:, :, 0].astype(f32)
    zpix = ((pv + f32(1.0)) * f32(0.5) * f32(NZ - 1)).astype(f32)
    z0f = np.floor(zpix)
    wz1 = (zpix - z0f).astype(f32)
    z0 = z0f.astype(np.int64)
    assert z0.min() >= 0 and z0.max() + 1 <= NZ - 1
    wz0 = (f32(1.0) - wz1).astype(f32)
    wz = np.zeros((NY, NZ, NV), dtype=f32)
    for v in range(NV):
        wz[kg[:, 0], z0[:, v], v] += wz0[:, v]
        wz[kg[:, 0], z0[:, v] + 1, v] += wz1[:, v]
    return dict(idx1=idx1, w1=w1, idx2=idx2, w2=w2, wz=wz)


_TABLES = None
_NC1 = None
_NC2 = None


def _get_tables():
    global _TABLES
    if _TABLES is None:
        _TABLES = _host_tables()
    return _TABLES


def _bcast_inner(ap, n):
    return bass.AP(ap.tensor, ap.offset, list(ap.ap) + [[0, n]])


def _build_stage1():
    """rv[a,k,x,z] = sum_st w1[a,k,st,x] * g1[a,k,(4x+st)*8+z]."""
    nc = bacc.Bacc(None, target_bir_lowering=False)
    dt = mybir.dt.float32
    g1p = nc.declare_dram_parameter("g1", [A_PER_CORE, NY, 4 * NX * NZ], dt, isOutput=False)
    w1p = nc.declare_dram_parameter("w1", [A_PER_CORE, NY, 4 * NX], dt, isOutput=False)
    rvp = nc.declare_dram_parameter("rv", [A_PER_CORE, NY, NX * NZ], dt, isOutput=True)
    NB = NY // 128
    with TileContext(nc) as tc:
        with (
            tc.tile_pool(name="work", bufs=2) as wpool,
            tc.tile_pool(name="big", bufs=2) as bpool,
        ):
            for a in range(A_PER_CORE):
                for b in range(NB):
                    sl = slice(128 * b, 128 * (b + 1))
                    g1 = bpool.tile([128, 4 * NX * NZ], dt, tag="g1")
                    w1t = wpool.tile([128, 4 * NX], dt, tag="w1t")
                    rv = wpool.tile([128, NX * NZ], dt, tag="rv")
                    tmp = bpool.tile([128, NX * NZ], dt, tag="tmp")
                    nc.sync.dma_start(out=g1[:], in_=g1p[a, sl, :])
                    nc.sync.dma_start(out=w1t[:], in_=w1p[a, sl, :])
                    g1_3d = g1[:].rearrange("p (x q) -> p x q", q=4 * NZ)
                    w1_3d = w1t[:].rearrange("p (st x) -> p st x", st=4)
                    rv_3d = rv[:].rearrange("p (x z) -> p x z", z=NZ)
                    tmp_3d = tmp[:].rearrange("p (x z) -> p x z", z=NZ)
                    for st in range(4):
                        tap = g1_3d[:, :, NZ * st:NZ * (st + 1)]
                        wb = _bcast_inner(w1_3d[:, st, :], NZ)
                        if st == 0:
                            nc.vector.tensor_tensor(out=rv_3d, in0=tap, in1=wb,
                                                    op=mybir.AluOpType.mult)
                        else:
                            nc.vector.tensor_tensor(out=tmp_3d, in0=tap, in1=wb,
                                                    op=mybir.AluOpType.mult)
                            nc.vector.tensor_tensor(out=rv_3d, in0=rv_3d, in1=tmp_3d,
                                                    op=mybir.AluOpType.add)
                    nc.sync.dma_start(out=rvp[a, sl, :], in_=rv[:])
    nc.finalize()
    return nc


def _build_stage2():
    """proj[a,v,u] = DIST * sum_{k,z} wz[k,z,v] *
         (sum_t w2[k,2u+t] * g2[a,k,(2u+t)*8+z])"""
    nc = bacc.Bacc(None, target_bir_lowering=False)
    dt = mybir.dt.float32
    g2p = nc.declare_dram_parameter("g2", [A_PER_CORE, NY, 2 * NU * NZ], dt, isOutput=False)
    w2p = nc.declare_dram_parameter("w2", [NY, 2 * NU], dt, isOutput=False)
    wzp = nc.declare_dram_parameter("wz", [NY, NZ * NV], dt, isOutput=False)
    distp = nc.declare_dram_parameter("dist", [NV, NU], dt, isOutput=False)
    outp = nc.declare_dram_parameter("out", [A_PER_CORE, NV, NU], dt, isOutput=True)
    NB = NY // 128
    with TileContext(nc) as tc:
        with (
            tc.tile_pool(name="const", bufs=1) as cpool,
            tc.tile_pool(name="work", bufs=2) as wpool,
            tc.tile_pool(name="big", bufs=2) as bpool,
            tc.tile_pool(name="psum", bufs=2, space="PSUM") as ppool,
        ):
            w2_sb = [cpool.tile([128, 2 * NU], dt, tag=f"w2_{b}", name=f"w2s_{b}")
                     for b in range(NB)]
            wz_sb = [cpool.tile([128, NZ * NV], dt, tag=f"wz_{b}", name=f"wzs_{b}")
                     for b in range(NB)]
            dist_sb = cpool.tile([NV, NU], dt, tag="dist")
            for b in range(NB):
                sl = slice(128 * b, 128 * (b + 1))
                nc.sync.dma_start(out=w2_sb[b][:], in_=w2p[sl, :])
                nc.sync.dma_start(out=wz_sb[b][:], in_=wzp[sl, :])
            nc.sync.dma_start(out=dist_sb[:], in_=distp[:])
            for a in range(A_PER_CORE):
                psum = ppool.tile([NV, NU], dt, tag="acc")
                first = True
                for b in range(NB):
                    sl = slice(128 * b, 128 * (b + 1))
                    g2 = bpool.tile([128, 2 * NU * NZ], dt, tag="g2")
                    tt = bpool.tile([128, NZ * NU], dt, tag="tt")
                    t2 = bpool.tile([128, NZ * NU], dt, tag="t2")
                    nc.sync.dma_start(out=g2[:], in_=g2p[a, sl, :])
                    # T[p, z*NU+u] = sum_t w2[p, 2u+t] * g2[p, (2u+t)*8+z]
                    g2_zu = g2[:].rearrange("p (u q) -> p q u", q=2 * NZ)
                    w2_tu = w2_sb[b][:].rearrange("p (u t) -> p t u", t=2)
                    tt_3d = tt[:].rearrange("p (z u) -> p z u", z=NZ)
                    t2_3d = t2[:].rearrange("p (z u) -> p z u", z=NZ)
                    wb0 = bass.AP(w2_tu.tensor, w2_tu.offset,
                                  [w2_tu.ap[0], [0, NZ], w2_tu.ap[2]])
                    wb1 = bass.AP(w2_tu.tensor, w2_tu.offset + w2_tu.ap[1][0],
                                  [w2_tu.ap[0], [0, NZ], w2_tu.ap[2]])
                    nc.vector.tensor_tensor(out=tt_3d, in0=g2_zu[:, 0:NZ, :], in1=wb0,
                                            op=mybir.AluOpType.mult)
                    nc.vector.tensor_tensor(out=t2_3d, in0=g2_zu[:, NZ:2 * NZ, :], in1=wb1,
                                            op=mybir.AluOpType.mult)
                    nc.vector.tensor_tensor(out=tt_3d, in0=tt_3d, in1=t2_3d,
                                            op=mybir.AluOpType.add)
                    wz_3d = wz_sb[b][:].rearrange("p (z v) -> p z v", z=NZ)
                    for z in range(NZ):
                        nc.tensor.matmul(
                            out=psum[:],
                            lhsT=wz_3d[:, z, :],
                            rhs=tt_3d[:, z, :],
                            start=first,
                            stop=(b == NB - 1 and z == NZ - 1),
                        )
                        first = False
                res = wpool.tile([NV, NU], dt, tag="res")
                nc.vector.tensor_tensor(out=res[:], in0=psum[:], in1=dist_sb[:],
                                        op=mybir.AluOpType.mult)
                nc.sync.dma_start(out=outp[a, :, :], in_=res[:])
    nc.finalize()
    return nc


def _get_ncs():
    global _NC1, _NC2
    if _NC1 is None:
        _NC1 = _build_stage1()
        _NC2 = _build_stage2()
    return _NC1, _NC2


def kernel(x: np.ndarray) -> np.ndarray:
    x = np.asarray(x, dtype=np.float32)
    B = x.shape[0]
    assert x.shape == (B, NZ, NY, NX) and B == 1
    t = _get_tables()
    nc1, nc2 = _get_ncs()
    volT = np.ascontiguousarray(x[0].transpose(1, 2, 0).reshape(NY * NX, NZ))

    # ---- launch 1: rotation tap-reduce (host stages the tap gather) ----
    in_maps1 = []
    for c in range(N_CORES):
        a0 = c * A_PER_CORE
        g1 = volT[t["idx1"][a0:a0 + A_PER_CORE]]          # [A, NY, 4NX, NZ]
        in_maps1.append({
            "g1": np.ascontiguousarray(g1.reshape(A_PER_CORE, NY, 4 * NX * NZ)),
            "w1": np.ascontiguousarray(
                t["w1"][a0:a0 + A_PER_CORE].reshape(A_PER_CORE, NY, 4 * NX)),
        })
    res1 = run_bass_kernel_spmd(nc1, in_maps1, core_ids=list(range(N_CORES)))

    # ---- launch 2: detector interp + projection sum ----
    w2_flat = np.ascontiguousarray(t["w2"].reshape(NY, 2 * NU))
    wz_flat = np.ascontiguousarray(t["wz"].reshape(NY, NZ * NV))
    in_maps2 = []
    for c in range(N_CORES):
        rv = res1.results[c]["rv"].reshape(A_PER_CORE, NY * NX, NZ)
        g2 = rv[:, t["idx2"], :]                          # [A, NY, 2NU, NZ]
        in_maps2.append({
            "g2": np.ascontiguousarray(g2.reshape(A_PER_CORE, NY, 2 * NU * NZ)),
            "w2": w2_flat,
            "wz": wz_flat,
            "dist": DIST,
        })
    res2 = run_bass_kernel_spmd(nc2, in_maps2, core_ids=list(range(N_CORES)))

    out = np.zeros((1, NA, NV, NU), dtype=np.float32)
    for c in range(N_CORES):
        out[0, c * A_PER_CORE:(c + 1) * A_PER_CORE] = res2.results[c]["out"]
    return out


if __name__ == "__main__":
    xv = np.random.default_rng(0).standard_normal((1, NZ, NY, NX)).astype(np.float32)
    y = kernel(xv)
    print("out", y.shape, y.dtype, "finite:", np.isfinite(y).all())


# revision 6
# speedup vs baseline: 1.0450x; 1.0450x over previous
"""FDK cone-beam forward projector on 8 trn2 NeuronCores (Bass).

Sharding: angle axis (64 angles -> 8 per core), volume replicated.

Device pipeline (two Bass launches, all arithmetic on device):
  launch 1 (per angle): 4-tap bilinear rotation reduction on DVE:
      rv[k,x,z] = sum_st w1[st,k,x] * G1[k, slot(x,st), z]
  launch 2 (per angle): detector x-interp on DVE + z-interp / y-sum on PE
      (PSUM-accumulated matmuls) + DIST scale.
The bilinear tap index tables are precomputed on host in float32 arithmetic
mirroring the reference exactly; the host applies the (pure data-movement)
tap gathers when staging each launch's inputs.
"""
import sys

sys.path.insert(0, "/opt/trn_rl_repo")

import numpy as np
import concourse.bass as bass
import concourse.bacc as bacc
import concourse.mybir as mybir
from concourse.bass_utils import run_bass_kernel_spmd
from concourse.tile import TileContext

# ---- geometry constants (mirror reference) ----
NA = 64
NZ, NY, NX = 8, 256, 256
NU, NV = 512, 8
DSD, DSO = 1085.6, 595.0
FOV, DZ = 500.0, 1.0
DU, DV = 1.0, 1.0
DX = DY = FOV / NX
HSX = DX * (NX / 2 - 0.5)
HSY = DY * (NY / 2 - 0.5)
HSZ = DZ * (NZ / 2 - 0.5)
ANGLES = np.arange(NA, dtype=np.float64) * (2.0 * np.pi / NA)
XS = (np.arange(NX) - NX / 2 + 0.5) * DX
YS = (np.arange(NY) - NY / 2 + 0.5) * DY
US = (np.arange(NU) - NU / 2 + 0.5) * DU
VS = (np.arange(NV) - NV / 2 + 0.5) * DV
XXN, YYN = np.meshgrid(XS / HSX, YS / HSY)
UU, VV = np.meshgrid(US, VS)
RATIO = (DSO - YS) / DSD
PU = UU[None] * RATIO[:, None, None] / HSX
PV = VV[None] * RATIO[:, None, None] / HSZ
DIST = (np.sqrt(DSD ** 2 + UU ** 2 + VV ** 2) / DSD * DY).astype(np.float32)

N_CORES = 8
A_PER_CORE = NA // N_CORES
f32 = np.float32


def _host_tables():
    cs_all = f32(np.cos(ANGLES))
    sn_all = f32(np.sin(ANGLES))
    xxn = XXN.astype(f32)
    yyn = YYN.astype(f32)
    idx1 = np.zeros((NA, NY, 4 * NX), dtype=np.int64)   # [a, k, slot=4x+st]
    w1 = np.zeros((NA, NY, 4, NX), dtype=f32)           # [a, k, st, x]
    for a in range(NA):
        cs, sn = -sn_all[a], cs_all[a]
        rx = (-xxn * sn + yyn * cs).astype(f32)
        ry = (xxn * cs + yyn * sn).astype(f32)
        xpix = ((rx + f32(1.0)) * f32(0.5) * f32(NX - 1)).astype(f32)
        ypix = ((ry + f32(1.0)) * f32(0.5) * f32(NY - 1)).astype(f32)
        x0f = np.floor(xpix)
        y0f = np.floor(ypix)
        wx1 = (xpix - x0f).astype(f32)
        wy1 = (ypix - y0f).astype(f32)
        x0 = x0f.astype(np.int64)
        y0 = y0f.astype(np.int64)
        wx0 = (f32(1.0) - wx1).astype(f32)
        wy0 = (f32(1.0) - wy1).astype(f32)
        for s in range(2):
            yi = y0 + s
            oky = (yi >= 0) & (yi < NY)
            yc = np.clip(yi, 0, NY - 1)
            wys = wy0 if s == 0 else wy1
            for t in range(2):
                xi = x0 + t
                okx = (xi >= 0) & (xi < NX)
                xc = np.clip(xi, 0, NX - 1)
                wxs = wx0 if t == 0 else wx1
                st = 2 * s + t
                w1[a, :, st, :] = (wys * wxs).astype(f32) * (oky & okx).astype(f32)
                idx1[a, :, st::4] = yc * NX + xc
    # stage 2 (angle-independent)
    pu = PU[:, 0, :].astype(f32)
    xpix2 = ((pu + f32(1.0)) * f32(0.5) * f32(NX - 1)).astype(f32)
    x20f = np.floor(xpix2)
    wx21 = (xpix2 - x20f).astype(f32)
    x20 = x20f.astype(np.int64)
    assert x20.min() >= 0 and x20.max() + 1 <= NX - 1
    wx20 = (f32(1.0) - wx21).astype(f32)
    idx2 = np.zeros((NY, 2 * NU), dtype=np.int64)       # [k, slot=2u+t]
    kg = np.arange(NY)[:, None]
    idx2[:, 0::2] = kg * NX + x20
    idx2[:, 1::2] = kg * NX + x20 + 1
    w2 = np.stack([wx20, wx21], axis=2)                 # [k, u, t]
    pv = PV[:, :, 0].astype(f32)
    zpix = ((pv + f32(1.0)) * f32(0.5) * f32(NZ - 1)).astype(f32)
    z0f = np.floor(zpix)
    wz1 = (zpix - z0f).astype(f32)
    z0 = z0f.astype(np.int64)
    assert z0.min() >= 0 and z0.max() + 1 <= NZ - 1
    wz0 = (f32(1.0) - wz1).astype(f32)
    wz = np.zeros((NY, NZ, NV), dtype=f32)
    for v in range(NV):
        wz[kg[:, 0], z0[:, v], v] += wz0[:, v]
        wz[kg[:, 0], z0[:, v] + 1, v] += wz1[:, v]
    return dict(idx1=idx1, w1=w1, idx2=idx2, w2=w2, wz=wz)


_TABLES = None
_NC1 = None
_NC2 = None


def _get_tables():
    global _TABLES
    if _TABLES is None:
        _TABLES = _host_tables()
    return _TABLES


def _bcast_inner(ap, n):
    return bass.AP(ap.tensor, ap.offset, list(ap.ap) + [[0, n]])


def _build_stage1():
    """rv[a,k,x,z] = sum_st w1[a,k,st,x] * g1[a,k,(4x+st)*8+z]."""
    nc = bacc.Bacc(None, target_bir_lowering=False)
    dt = mybir.dt.float32
    g1p = nc.declare_dram_parameter("g1", [A_PER_CORE, NY, 4 * NX * NZ], dt, isOutput=False)
    w1p = nc.declare_dram_parameter("w1", [A_PER_CORE, NY, 4 * NX], dt, isOutput=False)
    rvp = nc.declare_dram_parameter("rv", [A_PER_CORE, NY, NX * NZ], dt, isOutput=True)
    NB = NY // 128
    with TileContext(nc) as tc:
        with (
            tc.tile_pool(name="work", bufs=2) as wpool,
            tc.tile_pool(name="big", bufs=2) as bpool,
        ):
            for a in range(A_PER_CORE):
                for b in range(NB):
                    sl = slice(128 * b, 128 * (b + 1))
                    g1 = bpool.tile([128, 4 * NX * NZ], dt, tag="g1")
                    w1t = wpool.tile([128, 4 * NX], dt, tag="w1t")
                    rv = wpool.tile([128, NX * NZ], dt, tag="rv")
                    tmp = bpool.tile([128, NX * NZ], dt, tag="tmp")
                    nc.sync.dma_start(out=g1[:], in_=g1p[a, sl, :])
                    nc.sync.dma_start(out=w1t[:], in_=w1p[a, sl, :])
                    g1_3d = g1[:].rearrange("p (x q) -> p x q", q=4 * NZ)
                    w1_3d = w1t[:].rearrange("p (st x) -> p st x", st=4)
                    rv_3d = rv[:].rearrange("p (x z) -> p x z", z=NZ)
                    tmp_3d = tmp[:].rearrange("p (x z) -> p x z", z=NZ)
                    for st in range(4):
                        tap = g1_3d[:, :, NZ * st:NZ * (st + 1)]
                        wb = _bcast_inner(w1_3d[:, st, :], NZ)
                        if st == 0:
                            nc.vector.tensor_tensor(out=rv_3d, in0=tap, in1=wb,
                                                    op=mybir.AluOpType.mult)
                        else:
                            nc.vector.tensor_tensor(out=tmp_3d, in0=tap, in1=wb,
                                                    op=mybir.AluOpType.mult)
                            nc.vector.tensor_tensor(out=rv_3d, in0=rv_3d, in1=tmp_3d,
                                                    op=mybir.AluOpType.add)
                    nc.sync.dma_start(out=rvp[a, sl, :], in_=rv[:])
    nc.finalize()
    return nc


def _build_stage2():
    """proj[a,v,u] = DIST * sum_{k,z} wz[k,z,v] *
         (sum_t w2[k,2u+t] * g2[a,k,(2u+t)*8+z])"""
    nc = bacc.Bacc(None, target_bir_lowering=False)
    dt = mybir.dt.float32
    g2p = nc.declare_dram_parameter("g2", [A_PER_CORE, NY, 2 * NU * NZ], dt, isOutput=False)
    w2p = nc.declare_dram_parameter("w2", [NY, 2 * NU], dt, isOutput=False)
    wzp = nc.declare_dram_parameter("wz", [NY, NZ * NV], dt, isOutput=False)
    distp = nc.declare_dram_parameter("dist", [NV, NU], dt, isOutput=False)
    outp = nc.declare_dram_parameter("out", [A_PER_CORE, NV, NU], dt, isOutput=True)
    NB = NY // 128
    with TileContext(nc) as tc:
        with (
            tc.tile_pool(name="const", bufs=1) as cpool,
            tc.tile_pool(name="work", bufs=2) as wpool,
            tc.tile_pool(name="big", bufs=2) as bpool,
            tc.tile_pool(name="psum", bufs=2, space="PSUM") as ppool,
        ):
            w2_sb = [cpool.tile([128, 2 * NU], dt, tag=f"w2_{b}", name=f"w2s_{b}")
                     for b in range(NB)]
            wz_sb = [cpool.tile([128, NZ * NV], dt, tag=f"wz_{b}", name=f"wzs_{b}")
                     for b in range(NB)]
            dist_sb = cpool.tile([NV, NU], dt, tag="dist")
            for b in range(NB):
                sl = slice(128 * b, 128 * (b + 1))
                nc.sync.dma_start(out=w2_sb[b][:], in_=w2p[sl, :])
                nc.sync.dma_start(out=wz_sb[b][:], in_=wzp[sl, :])
            nc.sync.dma_start(out=dist_sb[:], in_=distp[:])
            for a in range(A_PER_CORE):
                psum = ppool.tile([NV, NU], dt, tag="acc")
                first = True
                for b in range(NB):
                    sl = slice(128 * b, 128 * (b + 1))
                    g2 = bpool.tile([128, 2 * NU * NZ], dt, tag="g2")
                    tt = bpool.tile([128, NZ * NU], dt, tag="tt")
                    t2 = bpool.tile([128, NZ * NU], dt, tag="t2")
                    nc.sync.dma_start(out=g2[:], in_=g2p[a, sl, :])
                    # T[p, z*NU+u] = sum_t w2[p, 2u+t] * g2[p, (2u+t)*8+z]
                    g2_zu = g2[:].rearrange("p (u q) -> p q u", q=2 * NZ)
                    w2_tu = w2_sb[b][:].rearrange("p (u t) -> p t u", t=2)
                    tt_3d = tt[:].rearrange("p (z u) -> p z u", z=NZ)
                    t2_3d = t2[:].rearrange("p (z u) -> p z u", z=NZ)
                    wb0 = bass.AP(w2_tu.tensor, w2_tu.offset,
                                  [w2_tu.ap[0], [0, NZ], w2_tu.ap[2]])
                    wb1 = bass.AP(w2_tu.tensor, w2_tu.offset + w2_tu.ap[1][0],
                                  [w2_tu.ap[0], [0, NZ], w2_tu.ap[2]])
                    nc.vector.tensor_tensor(out=tt_3d, in0=g2_zu[:, 0:NZ, :], in1=wb0,
                                            op=mybir.AluOpType.mult)
                    nc.vector.tensor_tensor(out=t2_3d, in0=g2_zu[:, NZ:2 * NZ, :], in1=wb1,
                                            op=mybir.AluOpType.mult)
                    nc.vector.tensor_tensor(out=tt_3d, in0=tt_3d, in1=t2_3d,
                                            op=mybir.AluOpType.add)
                    wz_3d = wz_sb[b][:].rearrange("p (z v) -> p z v", z=NZ)
                    for z in range(NZ):
                        nc.tensor.matmul(
                            out=psum[:],
                            lhsT=wz_3d[:, z, :],
                            rhs=tt_3d[:, z, :],
                            start=first,
                            stop=(b == NB - 1 and z == NZ - 1),
                        )
                        first = False
                res = wpool.tile([NV, NU], dt, tag="res")
                nc.vector.tensor_tensor(out=res[:], in0=psum[:], in1=dist_sb[:],
                                        op=mybir.AluOpType.mult)
                nc.sync.dma_start(out=outp[a, :, :], in_=res[:])
    nc.finalize()
    return nc


def _get_ncs():
    global _NC1, _NC2
    if _NC1 is None:
        _NC1 = _build_stage1()
        _NC2 = _build_stage2()
    return _NC1, _NC2


LAST_TIMING = {}


def kernel(x: np.ndarray) -> np.ndarray:
    import time as _time
    x = np.asarray(x, dtype=np.float32)
    B = x.shape[0]
    assert x.shape == (B, NZ, NY, NX) and B == 1
    t = _get_tables()
    nc1, nc2 = _get_ncs()
    volT = np.ascontiguousarray(x[0].transpose(1, 2, 0).reshape(NY * NX, NZ))

    # ---- launch 1: rotation tap-reduce (host stages the tap gather) ----
    t0 = _time.perf_counter()
    w1_flat = t["w1"].reshape(NA, NY, 4 * NX)
    idx1 = t["idx1"]
    in_maps1 = []
    for c in range(N_CORES):
        a0 = c * A_PER_CORE
        g1 = np.take(volT, idx1[a0:a0 + A_PER_CORE], axis=0)
        in_maps1.append({
            "g1": g1.reshape(A_PER_CORE, NY, 4 * NX * NZ),
            "w1": w1_flat[a0:a0 + A_PER_CORE],
        })
    t1 = _time.perf_counter()
    res1 = run_bass_kernel_spmd(nc1, in_maps1, core_ids=list(range(N_CORES)))
    t2 = _time.perf_counter()

    # ---- launch 2: detector interp + projection sum ----
    w2_flat = t["w2"].reshape(NY, 2 * NU)
    wz_flat = t["wz"].reshape(NY, NZ * NV)
    in_maps2 = []
    for c in range(N_CORES):
        rv = res1.results[c]["rv"].reshape(A_PER_CORE, NY * NX, NZ)
        g2 = np.take(rv, t["idx2"], axis=1)               # [A, NY, 2NU, NZ]
        in_maps2.append({
            "g2": g2.reshape(A_PER_CORE, NY, 2 * NU * NZ),
            "w2": w2_flat,
            "wz": wz_flat,
            "dist": DIST,
        })
    t3 = _time.perf_counter()
    res2 = run_bass_kernel_spmd(nc2, in_maps2, core_ids=list(range(N_CORES)))
    t4 = _time.perf_counter()

    out = np.zeros((1, NA, NV, NU), dtype=np.float32)
    for c in range(N_CORES):
        out[0, c * A_PER_CORE:(c + 1) * A_PER_CORE] = res2.results[c]["out"]
    LAST_TIMING.update(host_gather1=t1 - t0, launch1=t2 - t1,
                       host_gather2=t3 - t2, launch2=t4 - t3)
    return out


if __name__ == "__main__":
    xv = np.random.default_rng(0).standard_normal((1, NZ, NY, NX)).astype(np.float32)
    y = kernel(xv)
    print("out", y.shape, y.dtype, "finite:", np.isfinite(y).all())


# revision 7
# speedup vs baseline: 1.0803x; 1.0337x over previous
"""FDK cone-beam forward projector on 8 trn2 NeuronCores (Bass).

Sharding: angle axis (64 angles -> 8 per core), volume replicated.

Device pipeline (two Bass launches, all arithmetic on device):
  launch 1 (per angle): 4-tap bilinear rotation reduction on DVE:
      rv[k,x,z] = sum_st w1[st,k,x] * G1[k, slot(x,st), z]
  launch 2 (per angle): detector x-interp on DVE + z-interp / y-sum on PE
      (PSUM-accumulated matmuls) + DIST scale.
The bilinear tap index tables are precomputed on host in float32 arithmetic
mirroring the reference exactly; the host applies the (pure data-movement)
tap gathers when staging each launch's inputs.
"""
import sys

sys.path.insert(0, "/opt/trn_rl_repo")

import numpy as np
import concourse.bass as bass
import concourse.bacc as bacc
import concourse.mybir as mybir
from concourse.bass_utils import run_bass_kernel_spmd
from concourse.tile import TileContext

# ---- geometry constants (mirror reference) ----
NA = 64
NZ, NY, NX = 8, 256, 256
NU, NV = 512, 8
DSD, DSO = 1085.6, 595.0
FOV, DZ = 500.0, 1.0
DU, DV = 1.0, 1.0
DX = DY = FOV / NX
HSX = DX * (NX / 2 - 0.5)
HSY = DY * (NY / 2 - 0.5)
HSZ = DZ * (NZ / 2 - 0.5)
ANGLES = np.arange(NA, dtype=np.float64) * (2.0 * np.pi / NA)
XS = (np.arange(NX) - NX / 2 + 0.5) * DX
YS = (np.arange(NY) - NY / 2 + 0.5) * DY
US = (np.arange(NU) - NU / 2 + 0.5) * DU
VS = (np.arange(NV) - NV / 2 + 0.5) * DV
XXN, YYN = np.meshgrid(XS / HSX, YS / HSY)
UU, VV = np.meshgrid(US, VS)
RATIO = (DSO - YS) / DSD
PU = UU[None] * RATIO[:, None, None] / HSX
PV = VV[None] * RATIO[:, None, None] / HSZ
DIST = (np.sqrt(DSD ** 2 + UU ** 2 + VV ** 2) / DSD * DY).astype(np.float32)

N_CORES = 8
A_PER_CORE = NA // N_CORES
f32 = np.float32


def _host_tables():
    cs_all = f32(np.cos(ANGLES))
    sn_all = f32(np.sin(ANGLES))
    xxn = XXN.astype(f32)
    yyn = YYN.astype(f32)
    idx1 = np.zeros((NA, NY, 4 * NX), dtype=np.int64)   # [a, k, slot=4x+st]
    w1 = np.zeros((NA, NY, 4, NX), dtype=f32)           # [a, k, st, x]
    for a in range(NA):
        cs, sn = -sn_all[a], cs_all[a]
        rx = (-xxn * sn + yyn * cs).astype(f32)
        ry = (xxn * cs + yyn * sn).astype(f32)
        xpix = ((rx + f32(1.0)) * f32(0.5) * f32(NX - 1)).astype(f32)
        ypix = ((ry + f32(1.0)) * f32(0.5) * f32(NY - 1)).astype(f32)
        x0f = np.floor(xpix)
        y0f = np.floor(ypix)
        wx1 = (xpix - x0f).astype(f32)
        wy1 = (ypix - y0f).astype(f32)
        x0 = x0f.astype(np.int64)
        y0 = y0f.astype(np.int64)
        wx0 = (f32(1.0) - wx1).astype(f32)
        wy0 = (f32(1.0) - wy1).astype(f32)
        for s in range(2):
            yi = y0 + s
            oky = (yi >= 0) & (yi < NY)
            yc = np.clip(yi, 0, NY - 1)
            wys = wy0 if s == 0 else wy1
            for t in range(2):
                xi = x0 + t
                okx = (xi >= 0) & (xi < NX)
                xc = np.clip(xi, 0, NX - 1)
                wxs = wx0 if t == 0 else wx1
                st = 2 * s + t
                w1[a, :, st, :] = (wys * wxs).astype(f32) * (oky & okx).astype(f32)
                idx1[a, :, st::4] = yc * NX + xc
    # stage 2 (angle-independent)
    pu = PU[:, 0, :].astype(f32)
    xpix2 = ((pu + f32(1.0)) * f32(0.5) * f32(NX - 1)).astype(f32)
    x20f = np.floor(xpix2)
    wx21 = (xpix2 - x20f).astype(f32)
    x20 = x20f.astype(np.int64)
    assert x20.min() >= 0 and x20.max() + 1 <= NX - 1
    wx20 = (f32(1.0) - wx21).astype(f32)
    idx2 = np.zeros((NY, 2 * NU), dtype=np.int64)       # [k, slot=2u+t]
    kg = np.arange(NY)[:, None]
    idx2[:, 0::2] = kg * NX + x20
    idx2[:, 1::2] = kg * NX + x20 + 1
    w2 = np.stack([wx20, wx21], axis=2)                 # [k, u, t]
    pv = PV[:, :, 0].astype(f32)
    zpix = ((pv + f32(1.0)) * f32(0.5) * f32(NZ - 1)).astype(f32)
    z0f = np.floor(zpix)
    wz1 = (zpix - z0f).astype(f32)
    z0 = z0f.astype(np.int64)
    assert z0.min() >= 0 and z0.max() + 1 <= NZ - 1
    wz0 = (f32(1.0) - wz1).astype(f32)
    wz = np.zeros((NY, NZ, NV), dtype=f32)
    for v in range(NV):
        wz[kg[:, 0], z0[:, v], v] += wz0[:, v]
        wz[kg[:, 0], z0[:, v] + 1, v] += wz1[:, v]
    return dict(idx1=idx1, w1=w1, idx2=idx2, w2=w2, wz=wz)


_TABLES = None
_NC1 = None
_NC2 = None


def _get_tables():
    global _TABLES
    if _TABLES is None:
        _TABLES = _host_tables()
    return _TABLES


def _bcast_inner(ap, n):
    return bass.AP(ap.tensor, ap.offset, list(ap.ap) + [[0, n]])


def _build_stage1():
    """rv[a,k,x,z] = sum_st w1[a,k,st,x] * g1[a,k,(4x+st)*8+z]."""
    nc = bacc.Bacc(None, target_bir_lowering=False)
    dt = mybir.dt.float32
    g1p = nc.declare_dram_parameter("g1", [A_PER_CORE, NY, 4 * NX * NZ], dt, isOutput=False)
    w1p = nc.declare_dram_parameter("w1", [A_PER_CORE, NY, 4 * NX], dt, isOutput=False)
    rvp = nc.declare_dram_parameter("rv", [A_PER_CORE, NY, NX * NZ], dt, isOutput=True)
    NB = NY // 128
    with TileContext(nc) as tc:
        with (
            tc.tile_pool(name="work", bufs=2) as wpool,
            tc.tile_pool(name="big", bufs=2) as bpool,
        ):
            for a in range(A_PER_CORE):
                for b in range(NB):
                    sl = slice(128 * b, 128 * (b + 1))
                    g1 = bpool.tile([128, 4 * NX * NZ], dt, tag="g1")
                    w1t = wpool.tile([128, 4 * NX], dt, tag="w1t")
                    rv = wpool.tile([128, NX * NZ], dt, tag="rv")
                    tmp = bpool.tile([128, NX * NZ], dt, tag="tmp")
                    nc.sync.dma_start(out=g1[:], in_=g1p[a, sl, :])
                    nc.sync.dma_start(out=w1t[:], in_=w1p[a, sl, :])
                    g1_3d = g1[:].rearrange("p (x q) -> p x q", q=4 * NZ)
                    w1_3d = w1t[:].rearrange("p (st x) -> p st x", st=4)
                    rv_3d = rv[:].rearrange("p (x z) -> p x z", z=NZ)
                    tmp_3d = tmp[:].rearrange("p (x z) -> p x z", z=NZ)
                    for st in range(4):
                        tap = g1_3d[:, :, NZ * st:NZ * (st + 1)]
                        wb = _bcast_inner(w1_3d[:, st, :], NZ)
                        if st == 0:
                            nc.vector.tensor_tensor(out=rv_3d, in0=tap, in1=wb,
                                                    op=mybir.AluOpType.mult)
                        else:
                            nc.vector.tensor_tensor(out=tmp_3d, in0=tap, in1=wb,
                                                    op=mybir.AluOpType.mult)
                            nc.vector.tensor_tensor(out=rv_3d, in0=rv_3d, in1=tmp_3d,
                                                    op=mybir.AluOpType.add)
                    nc.sync.dma_start(out=rvp[a, sl, :], in_=rv[:])
    nc.finalize()
    return nc


def _build_stage2():
    """proj[a,v,u] = DIST * sum_{k,z} wz[k,z,v] *
         (sum_t w2[k,2u+t] * g2[a,k,(2u+t)*8+z])"""
    nc = bacc.Bacc(None, target_bir_lowering=False)
    dt = mybir.dt.float32
    g2p = nc.declare_dram_parameter("g2", [A_PER_CORE, NY, 2 * NU * NZ], dt, isOutput=False)
    w2p = nc.declare_dram_parameter("w2", [NY, 2 * NU], dt, isOutput=False)
    wzp = nc.declare_dram_parameter("wz", [NY, NZ * NV], dt, isOutput=False)
    distp = nc.declare_dram_parameter("dist", [NV, NU], dt, isOutput=False)
    outp = nc.declare_dram_parameter("out", [A_PER_CORE, NV, NU], dt, isOutput=True)
    NB = NY // 128
    with TileContext(nc) as tc:
        with (
            tc.tile_pool(name="const", bufs=1) as cpool,
            tc.tile_pool(name="work", bufs=2) as wpool,
            tc.tile_pool(name="big", bufs=2) as bpool,
            tc.tile_pool(name="psum", bufs=2, space="PSUM") as ppool,
        ):
            w2_sb = [cpool.tile([128, 2 * NU], dt, tag=f"w2_{b}", name=f"w2s_{b}")
                     for b in range(NB)]
            wz_sb = [cpool.tile([128, NZ * NV], dt, tag=f"wz_{b}", name=f"wzs_{b}")
                     for b in range(NB)]
            dist_sb = cpool.tile([NV, NU], dt, tag="dist")
            for b in range(NB):
                sl = slice(128 * b, 128 * (b + 1))
                nc.sync.dma_start(out=w2_sb[b][:], in_=w2p[sl, :])
                nc.sync.dma_start(out=wz_sb[b][:], in_=wzp[sl, :])
            nc.sync.dma_start(out=dist_sb[:], in_=distp[:])
            for a in range(A_PER_CORE):
                psum = ppool.tile([NV, NU], dt, tag="acc")
                first = True
                for b in range(NB):
                    sl = slice(128 * b, 128 * (b + 1))
                    g2 = bpool.tile([128, 2 * NU * NZ], dt, tag="g2")
                    tt = bpool.tile([128, NZ * NU], dt, tag="tt")
                    t2 = bpool.tile([128, NZ * NU], dt, tag="t2")
                    nc.sync.dma_start(out=g2[:], in_=g2p[a, sl, :])
                    # T[p, z*NU+u] = sum_t w2[p, 2u+t] * g2[p, (2u+t)*8+z]
                    g2_zu = g2[:].rearrange("p (u q) -> p q u", q=2 * NZ)
                    w2_tu = w2_sb[b][:].rearrange("p (u t) -> p t u", t=2)
                    tt_3d = tt[:].rearrange("p (z u) -> p z u", z=NZ)
                    t2_3d = t2[:].rearrange("p (z u) -> p z u", z=NZ)
                    wb0 = bass.AP(w2_tu.tensor, w2_tu.offset,
                                  [w2_tu.ap[0], [0, NZ], w2_tu.ap[2]])
                    wb1 = bass.AP(w2_tu.tensor, w2_tu.offset + w2_tu.ap[1][0],
                                  [w2_tu.ap[0], [0, NZ], w2_tu.ap[2]])
                    nc.vector.tensor_tensor(out=tt_3d, in0=g2_zu[:, 0:NZ, :], in1=wb0,
                                            op=mybir.AluOpType.mult)
                    nc.vector.tensor_tensor(out=t2_3d, in0=g2_zu[:, NZ:2 * NZ, :], in1=wb1,
                                            op=mybir.AluOpType.mult)
                    nc.vector.tensor_tensor(out=tt_3d, in0=tt_3d, in1=t2_3d,
                                            op=mybir.AluOpType.add)
                    wz_3d = wz_sb[b][:].rearrange("p (z v) -> p z v", z=NZ)
                    for z in range(NZ):
                        nc.tensor.matmul(
                            out=psum[:],
                            lhsT=wz_3d[:, z, :],
                            rhs=tt_3d[:, z, :],
                            start=first,
                            stop=(b == NB - 1 and z == NZ - 1),
                        )
                        first = False
                res = wpool.tile([NV, NU], dt, tag="res")
                nc.vector.tensor_tensor(out=res[:], in0=psum[:], in1=dist_sb[:],
                                        op=mybir.AluOpType.mult)
                nc.sync.dma_start(out=outp[a, :, :], in_=res[:])
    nc.finalize()
    return nc


def _get_ncs():
    global _NC1, _NC2
    if _NC1 is None:
        _NC1 = _build_stage1()
        _NC2 = _build_stage2()
    return _NC1, _NC2


_RUNNERS = {}


def _cached_spmd(nc, in_maps):
    """run_bass_kernel_spmd equivalent with the shard_map jit traced once."""
    import jax
    from jax.sharding import Mesh, PartitionSpec
    from jax.experimental.shard_map import shard_map
    from concourse import bass2jax

    key = id(nc)
    if key not in _RUNNERS:
        bass2jax.install_neuronx_cc_hook()
        partition_name = (nc.partition_id_tensor.name
                          if nc.partition_id_tensor else None)
        in_names, out_names, out_avals, zero_shapes = [], [], [], []
        for alloc in nc.m.functions[0].allocations:
            if not isinstance(alloc, mybir.MemoryLocationSet):
                continue
            name = alloc.memorylocations[0].name
            if alloc.kind == "ExternalInput":
                if name != partition_name:
                    in_names.append(name)
            elif alloc.kind == "ExternalOutput":
                out_names.append(name)
                shape = tuple(alloc.tensor_shape)
                dtype = mybir.dt.np(alloc.dtype)
                out_avals.append(jax.core.ShapedArray(shape, dtype))
                zero_shapes.append((shape, dtype))
        all_in = list(in_names) + list(out_names)
        if partition_name is not None:
            all_in.append(partition_name)

        def _body(*args):
            operands = list(args)
            if partition_name is not None:
                operands.append(bass2jax.partition_id_tensor())
            return tuple(bass2jax._bass_exec_p.bind(
                *operands, out_avals=tuple(out_avals),
                in_names=tuple(all_in), out_names=tuple(out_names),
                lowering_input_output_aliases=(),
                sim_require_finite=True, sim_require_nnan=True, nc=nc))

        devices = jax.devices()[:N_CORES]
        mesh = Mesh(np.asarray(devices), ("core",))
        n_io = len(in_names) + len(out_names)
        sharded = jax.jit(
            shard_map(_body, mesh=mesh,
                      in_specs=(PartitionSpec("core"),) * n_io,
                      out_specs=(PartitionSpec("core"),) * len(out_names),
                      check_rep=False),
            keep_unused=True)
        _RUNNERS[key] = (sharded, in_names, out_names, zero_shapes)

    sharded, in_names, out_names, zero_shapes = _RUNNERS[key]
    concat_in = [np.concatenate([np.asarray(m[n]) for m in in_maps], axis=0)
                 for n in in_names]
    zeros = [np.zeros((N_CORES * s0[0], *s0[1:]), d) for s0, d in zero_shapes]
    outs = sharded(*concat_in, *zeros)
    outs = [np.asarray(o) for o in outs]
    per_core = []
    for c in range(N_CORES):
        per_core.append({
            n: outs[i].reshape(N_CORES, *zero_shapes[i][0])[c]
            for i, n in enumerate(out_names)})
    return per_core


LAST_TIMING = {}


def kernel(x: np.ndarray) -> np.ndarray:
    import time as _time
    x = np.asarray(x, dtype=np.float32)
    B = x.shape[0]
    assert x.shape == (B, NZ, NY, NX) and B == 1
    t = _get_tables()
    nc1, nc2 = _get_ncs()
    volT = np.ascontiguousarray(x[0].transpose(1, 2, 0).reshape(NY * NX, NZ))

    # ---- launch 1: rotation tap-reduce (host stages the tap gather) ----
    t0 = _time.perf_counter()
    w1_flat = t["w1"].reshape(NA, NY, 4 * NX)
    idx1 = t["idx1"]
    in_maps1 = []
    for c in range(N_CORES):
        a0 = c * A_PER_CORE
        g1 = np.take(volT, idx1[a0:a0 + A_PER_CORE], axis=0)
        in_maps1.append({
            "g1": g1.reshape(A_PER_CORE, NY, 4 * NX * NZ),
            "w1": w1_flat[a0:a0 + A_PER_CORE],
        })
    t1 = _time.perf_counter()
    try:
        res1 = _cached_spmd(nc1, in_maps1)
    except Exception:
        res1 = run_bass_kernel_spmd(
            nc1, in_maps1, core_ids=list(range(N_CORES))).results
    t2 = _time.perf_counter()

    # ---- launch 2: detector interp + projection sum ----
    w2_flat = t["w2"].reshape(NY, 2 * NU)
    wz_flat = t["wz"].reshape(NY, NZ * NV)
    in_maps2 = []
    for c in range(N_CORES):
        rv = res1[c]["rv"].reshape(A_PER_CORE, NY * NX, NZ)
        g2 = np.take(rv, t["idx2"], axis=1)               # [A, NY, 2NU, NZ]
        in_maps2.append({
            "g2": g2.reshape(A_PER_CORE, NY, 2 * NU * NZ),
            "w2": w2_flat,
            "wz": wz_flat,
            "dist": DIST,
        })
    t3 = _time.perf_counter()
    try:
        res2 = _cached_spmd(nc2, in_maps2)
    except Exception:
        res2 = run_bass_kernel_spmd(
            nc2, in_maps2, core_ids=list(range(N_CORES))).results
    t4 = _time.perf_counter()

    out = np.zeros((1, NA, NV, NU), dtype=np.float32)
    for c in range(N_CORES):
        out[0, c * A_PER_CORE:(c + 1) * A_PER_CORE] = res2[c]["out"]
    LAST_TIMING.update(host_gather1=t1 - t0, launch1=t2 - t1,
                       host_gather2=t3 - t2, launch2=t4 - t3)
    return out


if __name__ == "__main__":
    xv = np.random.default_rng(0).standard_normal((1, NZ, NY, NX)).astype(np.float32)
    y = kernel(xv)
    print("out", y.shape, y.dtype, "finite:", np.isfinite(y).all())
